# revision 26
# baseline (speedup 1.0000x reference)
"""ArcFace head on 8 TRN2 NeuronCores (Bass/Tile).

Model-parallel over classes: each of the 8 cores owns a 12500-class slice
of the 100000-class weight matrix and computes its (1024 x 12500) slice of
the logits; the host concatenates slices along the class dim.

Per-core device kernel:
  - normalize embeddings (64/||e|| folded in) and the weight slice
    (1/||w|| folded in), cast to bf16
  - (1024 x 512) @ (512 x 12500) matmul on TensorE, f32 accumulate
  - ArcFace margin: gather the label's weight row per sample
    (indirect DMA), compute cos(theta+m) per row in f32, scatter the
    corrected target logits into the output (indirect DMA, out-of-shard
    rows skipped via the bounds check)

Inputs are pre-arranged on the host (transposed weight slice for the
matmul operands, per-shard relabeled indices); all arithmetic of the op
itself runs on device.
"""

import math

import ml_dtypes
import numpy as np

import concourse.bacc as bacc
import concourse.bass as bass
import concourse.mybir as mybir
import concourse.tile as tile

# Problem constants (hardcoded per harness rules).
B = 1024  # batch
D = 512  # embedding dim
C = 100000  # num classes
NCORES = 8
CS = C // NCORES  # classes per core = 12500
P = 128  # partitions
KCH = D // P  # contraction chunks = 4
NB = B // P  # batch tiles = 8
CW = 500  # class window per matmul (<=512 psum bank, divides 12500)
NCW = CS // CW  # 25 class windows

SCALE = 64.0
MARGIN = 0.5
COS_M = math.cos(MARGIN)
SIN_M = math.sin(MARGIN)
TH = math.cos(math.pi - MARGIN)
MM = math.sin(math.pi - MARGIN) * MARGIN

F32 = mybir.dt.float32
BF16 = mybir.dt.bfloat16
I32 = mybir.dt.int32

OOB_SCATTER = 1 << 26  # out-of-shard sentinel for scatter offsets


def build_graph():
    nc = bacc.Bacc(
        "TRN2",
        target_bir_lowering=False,
        debug=False,
        num_devices=NCORES,
    )

    embT = nc.declare_dram_parameter("embT", [D, B], F32, isOutput=False)
    wT = nc.declare_dram_parameter("wT", [D, CS], BF16, isOutput=False)
    w_nat = nc.declare_dram_parameter("w_nat", [CS, D], F32, isOutput=False)
    emb = nc.declare_dram_parameter("emb", [B, D], F32, isOutput=False)
    gidx = nc.declare_dram_parameter("gidx", [P, NB], I32, isOutput=False)
    soff = nc.declare_dram_parameter("soff", [P, NB], I32, isOutput=False)
    out = nc.declare_dram_parameter("out", [B, CS], F32, isOutput=True)

    # DRAM views: partition p of contraction chunk k holds row k*128+p;
    # batch row b maps to (partition b%128, tile b//128).
    embT_r = embT[:].rearrange("(k p) b -> p k b", p=P)  # (128, 4, 1024)
    wT_r = wT[:].rearrange("(k p) c -> p k c", p=P)  # (128, 4, 12500)
    emb_r = emb[:].rearrange("(i p) d -> p i d", p=P)  # (128, 8, 512)
    out_r = out[:].rearrange("(i p) c -> p i c", p=P)  # (128, 8, 12500)
    out_flat = out[:].rearrange("a b -> (a b)")[:, None]  # (12.8M, 1)

    with tile.TileContext(nc) as tc:
        with (
            tc.tile_pool(name="const", bufs=1) as constp,
            tc.tile_pool(name="embp", bufs=1) as embp,
            tc.tile_pool(name="wstage", bufs=4) as wstage,
            tc.tile_pool(name="wnb", bufs=2) as wnbp,
            tc.tile_pool(name="wsq", bufs=2) as wsqp,
            tc.tile_pool(name="wnt", bufs=3) as wntp,
            tc.tile_pool(name="ostripe", bufs=2) as ostripep,
            tc.tile_pool(name="small", bufs=2) as smallp,
            tc.tile_pool(name="marg", bufs=1) as margp,
            tc.tile_pool(name="ps_main", bufs=3, space="PSUM") as ps_main,
            tc.tile_pool(name="ps_small", bufs=2, space="PSUM") as ps_small,
        ):
            # Constants.
            ones_col_bf = constp.tile([P, 1], BF16, tag="ones_col")
            nc.vector.memset(ones_col_bf[:], 1.0)

            # ---------- embedding prep: embT_n = 64 * emb.T / ||emb|| (bf16)
            embT_f = embp.tile([P, KCH, B], F32, tag="embT_f")
            nc.sync.dma_start(out=embT_f[:], in_=embT_r[:])
            emb2 = embp.tile([P, KCH, B], BF16, tag="emb2")
            nc.scalar.square(emb2[:], embT_f[:])
            # ||e||^2 per batch col via ones-matmul (partition reduction).
            eb_ps = []
            for h in range(2):
                pe = ps_small.tile([1, 512], F32, tag="ps_small")
                for k in range(KCH):
                    nc.tensor.matmul(
                        pe[:],
                        lhsT=ones_col_bf[:],
                        rhs=emb2[:, k, h * 512 : (h + 1) * 512],
                        start=(k == 0),
                        stop=(k == KCH - 1),
                    )
                eb_ps.append(pe)
            enorm = smallp.tile([1, B], F32, tag="enorm")
            for h in range(2):
                nc.scalar.sqrt(enorm[:, h * 512 : (h + 1) * 512], eb_ps[h][:])
            erec = smallp.tile([1, B], F32, tag="erec")
            escr = smallp.tile([1, B], F32, tag="escr")
            nc.vector.reciprocal_approx_accurate(erec[:], enorm[:], escr[:])
            nc.vector.tensor_scalar_mul(erec[:], erec[:], SCALE)
            ebb = embp.tile([P, B], F32, tag="ebb")
            nc.gpsimd.partition_broadcast(ebb[:], erec[:])
            embT_n = embp.tile([P, KCH, B], BF16, tag="embT_n")
            for k in range(KCH):
                nc.vector.tensor_mul(embT_n[:, k, :], embT_f[:, k, :], ebb[:])

            # ---------- margin path: corrected target logits per sample.
            # Data movement happens up front (sync/gpsimd queues); the
            # vector/scalar compute is emitted interleaved with the main
            # loop (one op per class window) so it never delays the
            # weight-prep chain feeding TensorE.
            emb_nat = margp.tile([P, NB, D], F32, tag="emb_nat")
            nc.sync.dma_start(out=emb_nat[:], in_=emb_r[:])
            gidx_t = margp.tile([P, NB], I32, tag="gidx_t")
            nc.sync.dma_start(out=gidx_t[:], in_=gidx[:])
            soff_t = margp.tile([P, NB], I32, tag="soff_t")
            nc.sync.dma_start(out=soff_t[:], in_=soff[:])

            wg = margp.tile([P, NB, D], F32, tag="wg")
            nc.gpsimd.memset(wg[:], 0.0)
            # gather w rows for in-shard labels; out-of-shard rows skipped
            # (one offset per partition per call — the layout the HW
            # indirect DGE path supports)
            for i in range(NB):
                nc.gpsimd.indirect_dma_start(
                    out=wg[:, i, :],
                    out_offset=None,
                    in_=w_nat[:],
                    in_offset=bass.IndirectOffsetOnAxis(
                        ap=gidx_t[:, i : i + 1], axis=0
                    ),
                    bounds_check=CS - 1,
                    oob_is_err=False,
                )

            mtmp = margp.tile([P, D], F32, tag="mtmp")
            en2 = margp.tile([P, NB], F32, tag="en2")
            gn2 = margp.tile([P, NB], F32, tag="gn2")
            dot = margp.tile([P, NB], F32, tag="dot")
            den = margp.tile([P, NB], F32, tag="den")
            rden = margp.tile([P, NB], F32, tag="rden")
            rscr = margp.tile([P, NB], F32, tag="rscr")
            cost = margp.tile([P, NB], F32, tag="cost")
            sint = margp.tile([P, NB], F32, tag="sint")
            cosm = margp.tile([P, NB], F32, tag="cosm")
            alt = margp.tile([P, NB], F32, tag="alt")
            mask = margp.tile([P, NB], mybir.dt.uint8, tag="mask")
            yv = margp.tile([P, NB], F32, tag="yv")
            X = mybir.AxisListType.X
            ADD = mybir.AluOpType.add

            def rowdot(a, b, acc, i):
                # acc[:, i] = sum_d a[:, i, :] * b[:, i, :], as two small ops
                def mul():
                    nc.vector.tensor_mul(mtmp[:], a[:, i, :], b[:, i, :])

                def red():
                    nc.vector.tensor_reduce(
                        acc[:, i : i + 1], mtmp[:, None, :], axis=X, op=ADD
                    )

                return [mul, red]

            margin_ops = []
            for a, b, acc in (
                (emb_nat, emb_nat, en2),
                (wg, wg, gn2),
                (emb_nat, wg, dot),
            ):
                for i in range(NB):
                    margin_ops += rowdot(a, b, acc, i)
            margin_ops += [
                # cos_t = dot / max(||e||*||w_label||, eps)
                lambda: nc.vector.tensor_mul(den[:], en2[:], gn2[:]),
                lambda: nc.scalar.sqrt(den[:], den[:]),
                lambda: nc.vector.tensor_scalar_max(den[:], den[:], 1e-12),
                lambda: nc.vector.reciprocal_approx_accurate(
                    rden[:], den[:], rscr[:]
                ),
                lambda: nc.vector.tensor_mul(cost[:], dot[:], rden[:]),
                # sin_t = sqrt(max(0, 1 - cos^2))
                lambda: nc.vector.tensor_mul(sint[:], cost[:], cost[:]),
                lambda: nc.vector.tensor_scalar(
                    out=sint[:],
                    in0=sint[:],
                    scalar1=-1.0,
                    scalar2=1.0,
                    op0=mybir.AluOpType.mult,
                    op1=ADD,
                ),
                lambda: nc.vector.tensor_scalar_max(sint[:], sint[:], 0.0),
                lambda: nc.scalar.sqrt(sint[:], sint[:]),
                # cos(t+m) = cos*COS_M - sin*SIN_M ; else branch: cos - MM
                lambda: nc.vector.tensor_scalar_mul(cosm[:], sint[:], -SIN_M),
                lambda: nc.vector.scalar_tensor_tensor(
                    out=cosm[:],
                    in0=cost[:],
                    scalar=COS_M,
                    in1=cosm[:],
                    op0=mybir.AluOpType.mult,
                    op1=ADD,
                ),
                lambda: nc.vector.tensor_scalar_add(alt[:], cost[:], -MM),
                lambda: nc.vector.tensor_single_scalar(
                    mask[:], cost[:], TH, mybir.AluOpType.is_gt
                ),
                lambda: nc.vector.select(yv[:], mask[:], cosm[:], alt[:]),
                lambda: nc.vector.tensor_scalar_mul(yv[:], yv[:], SCALE),
            ]

            # ---------- main loop over class windows, weight prep pipelined
            # one iteration ahead of the matmuls that consume it
            def prep(cw):
                csl = slice(cw * CW, (cw + 1) * CW)
                wt_f = wstage.tile([P, KCH, CW], BF16, tag="wt_f")
                nc.sync.dma_start(out=wt_f[:], in_=wT_r[:, :, csl])
                w2 = wsqp.tile([P, KCH, CW], BF16, tag="w2")
                nc.scalar.square(w2[:], wt_f[:])
                pn = ps_small.tile([1, 512], F32, tag="ps_small")
                for k in range(KCH):
                    nc.tensor.matmul(
                        pn[:, :CW],
                        lhsT=ones_col_bf[:],
                        rhs=w2[:, k, :],
                        start=(k == 0),
                        stop=(k == KCH - 1),
                    )
                rn = smallp.tile([1, CW], F32, tag="rn")
                nc.scalar.sqrt(rn[:], pn[:, :CW])
                rrec = smallp.tile([1, CW], F32, tag="rrec")
                rscrw = smallp.tile([1, CW], F32, tag="rscrw")
                nc.vector.reciprocal_approx_accurate(rrec[:], rn[:], rscrw[:])
                rrecb = smallp.tile([1, CW], BF16, tag="rrecb")
                nc.scalar.copy(rrecb[:], rrec[:])
                wnb = wnbp.tile([P, CW], BF16, tag="wnb")
                nc.gpsimd.partition_broadcast(wnb[:], rrecb[:])
                wnt = wntp.tile([P, KCH, CW], BF16, tag="wnt")
                nc.vector.tensor_mul(
                    wnt[:],
                    wt_f[:],
                    wnb[:, None, :].to_broadcast([P, KCH, CW]),
                )
                return wnt

            wnt_cur = prep(0)
            for cw in range(NCW):
                wnt_next = prep(cw + 1) if cw + 1 < NCW else None
                ostripe = ostripep.tile([P, NB, CW], F32, tag="ostripe")
                for half in range(NB // 2):
                    # pair of bank-aligned psum tiles drained in one op
                    po2 = ps_main.tile([P, 2, 512], F32, tag="ps_main")
                    for j in range(2):
                        bt = half * 2 + j
                        for k in range(KCH):
                            nc.tensor.matmul(
                                po2[:, j, :CW],
                                lhsT=embT_n[:, k, bt * P : (bt + 1) * P],
                                rhs=wnt_cur[:, k, :],
                                start=(k == 0),
                                stop=(k == KCH - 1),
                            )
                    if half % 2 == 0:
                        nc.scalar.copy(
                            ostripe[:, half * 2 : half * 2 + 2, :], po2[:, :, :CW]
                        )
                    else:
                        nc.vector.tensor_copy(
                            ostripe[:, half * 2 : half * 2 + 2, :], po2[:, :, :CW]
                        )
                # out-DMAs on the gpsimd (SWDGE) queue so they never block
                # the sync queue's input prefetch stream
                nc.gpsimd.dma_start(
                    out=out_r[:, :, cw * CW : (cw + 1) * CW], in_=ostripe[:]
                )
                for _ in range(3):
                    if margin_ops:
                        margin_ops.pop(0)()
                wnt_cur = wnt_next
            while margin_ops:
                margin_ops.pop(0)()

            # ---------- scatter corrected target logits (after main writes)
            for i in range(NB):
                nc.gpsimd.indirect_dma_start(
                    out=out_flat,
                    out_offset=bass.IndirectOffsetOnAxis(
                        ap=soff_t[:, i : i + 1], axis=0
                    ),
                    in_=yv[:, i : i + 1],
                    in_offset=None,
                    bounds_check=B * CS - 1,
                    oob_is_err=False,
                )

    nc.compile()
    return nc


def make_in_maps(embeddings, labels, weight):
    """Shard + lay out the inputs for the 8 cores."""
    emb = np.ascontiguousarray(embeddings, dtype=np.float32)
    embT = np.ascontiguousarray(emb.T)
    lab = np.asarray(labels).astype(np.int64)
    w = np.asarray(weight, dtype=np.float32)

    bidx = np.arange(B)
    p_of_b = bidx % P  # partition
    i_of_b = bidx // P  # batch tile

    in_maps = []
    for c in range(NCORES):
        lo = c * CS
        local = lab - lo
        in_shard = (local >= 0) & (local < CS)
        gidx = np.full((P, NB), CS, dtype=np.int32)  # CS -> OOB, skipped
        gidx[p_of_b, i_of_b] = np.where(in_shard, local, CS).astype(np.int32)
        soff = np.full((P, NB), OOB_SCATTER, dtype=np.int32)
        soff[p_of_b, i_of_b] = np.where(
            in_shard, bidx * CS + np.clip(local, 0, CS - 1), OOB_SCATTER
        ).astype(np.int32)
        wsh = w[lo : lo + CS]
        in_maps.append(
            {
                "embT": embT,
                "wT": np.ascontiguousarray(wsh.T).astype(ml_dtypes.bfloat16),
                "w_nat": np.ascontiguousarray(wsh),
                "emb": emb,
                "gidx": gidx,
                "soff": soff,
            }
        )
    return in_maps


_CACHED_NC = None


def _get_graph():
    global _CACHED_NC
    if _CACHED_NC is None:
        _CACHED_NC = build_graph()
    return _CACHED_NC


def kernel(embeddings, labels, weight):
    from concourse.bass_utils import run_bass_kernel_spmd

    nc = _get_graph()
    in_maps = make_in_maps(embeddings, labels, weight)
    res = run_bass_kernel_spmd(nc, in_maps, core_ids=list(range(NCORES)))
    return np.concatenate([res.results[i]["out"] for i in range(NCORES)], axis=1)


if __name__ == "__main__":
    nc = build_graph()
    print("graph built ok")
